# revision 60
# baseline (speedup 1.0000x reference)
"""Modulated deformable conv (DCNv2) on 8 trn2 NeuronCores, data-parallel over batch.

Engine-balanced tent-window formulation:
  out[o,i,j] = sum_k mask[k,ij] * sum_{ey,ex} T(dy[k,ij]-ey)*T(dx[k,ij]-ex)
               * V[k,o, i+ky-1+ey, j+kx-1+ex],   T(t) = relu(1-|t|)
  V[k,o] = sum_c w_reg[o,c,k] x[c]  (PE projection).

Per 8-row band:
- PE projects only the unshifted (sx=0) V triplet (2 matmuls/slot); the 12
  column-shifted variants are materialized by DMA via a DRAM staging row
  buffer whose edge rows are host-zeroed (partition-shifted SBUF-SBUF DMA and
  raw-AP reads of DMA-written tiles both break dependency tracking, so all
  SBUF-side DMA APs stay full-partition).
- VT2 is laid out [j; col, slot] so the MAC's innermost free dim is the band
  row i (stride 1): all-bf16 tensor_tensor ops hit DVE 2x_1p mode, and one
  raw-AP op covers all 3 ky taps (col+=64 and slot+=1 jointly).
- 63 merged (kx,ey,ex) terms/band: mults 31 DVE / 32 GPSIMD; accumulation:
  53 terms via PE identity-matmul PSUM accumulate (fp32, free ky-collapse),
  10 via DVE bf16 adds.
- offset/mask convs + transposes are emitted per-band one band ahead
  (rolling 4-row buffer), and VT2 double-buffers across bands, so all five
  engines + both DMA queues stay loaded from ~20us in.

CoreSim: 3.044 ms (previous session baseline) -> 0.779 ms; rel err 6.8e-3.
"""
import sys

sys.path.insert(0, "/opt/trn_rl_repo")

import numpy as np

import concourse.bass as bass
import concourse.mybir as mybir
import concourse.tile as tile
from concourse.bass_utils import run_bass_kernel_spmd

F32 = mybir.dt.float32
BF16 = mybir.dt.bfloat16
ALU = mybir.AluOpType
ACTF = mybir.ActivationFunctionType

H = W = 128
C = O = 64
KK = 9
PW = 134          # padded width/height, image at [3, 131)
R = 8             # output rows per band
NBANDS = H // R
SLOTS = R + 6     # source rows per band: image rows [i0-3, i0+R+3)
NCORES = 8

# (sx, kx) projection groups; ex = sx-kx+1 in [-2,2]
SXKX = [(sx, kx) for sx in range(-3, 4) for kx in range(3) if abs(sx - kx + 1) <= 2]
GOFF = {}
_off = 0
for sx, kx in SXKX:
    GOFF[(sx, kx)] = _off
    _off += 3 * O  # 3 ky taps x 64 outputs
GW = _off  # 2880 cols per slot
SXRANGE = {}
for sx in range(-3, 4):
    kxs = [kx for s2, kx in SXKX if s2 == sx]
    SXRANGE[sx] = (min(kxs), max(kxs))

TERMS2 = [
    (kx, ey, ex)
    for kx in range(3)
    for ey in (-2, -1, 0, 1, 2)
    for ex in (-2, -1, 0, 1, 2)
    if not (abs(ey) == 2 and abs(ex) == 2)
]  # 63 merged terms (3 ky taps each)

# engine split: mults on DVE vs GPSIMD; adds on PE-psum vs DVE
GPS_MULT = {t for i, t in enumerate(TERMS2) if i % 2 == 0}  # 32 terms
_gps_list = [t for t in TERMS2 if t in GPS_MULT]
DVE_ADD = set(_gps_list[:7])  # their adds stay off the PE
# remaining terms accumulate via PE psum


def _fix_multiwait(nc, max_waits=1):
    """This walrus build accepts at most one sync-wait per instruction; hoist
    extras onto same-engine NoOps inserted just before."""
    import bass_rust

    ctr = 0
    for f in nc.m.functions:
        for bb in f.blocks:
            insts = bb.instructions

            def nwaits(i):
                si = i.sync_info
                return len(si.on_wait) if si is not None else 0

            if not any(nwaits(i) > max_waits for i in insts):
                continue
            out = []
            for inst in insts:
                si = inst.sync_info
                waits = list(si.on_wait) if si is not None else []
                if len(waits) > max_waits:
                    extra, keep = waits[:-max_waits], waits[-max_waits:]
                    for j in range(0, len(extra), max_waits):
                        ctr += 1
                        nop = mybir.InstNoOp(name=f"WFIX-{ctr}", ins=[], outs=[])
                        nop.engine = inst.engine
                        nop.sync_info = bass_rust.SyncInfo(
                            on_wait=extra[j : j + max_waits], on_update=[]
                        )
                        out.append(nop)
                    inst.sync_info = bass_rust.SyncInfo(
                        on_wait=keep, on_update=list(si.on_update)
                    )
                out.append(inst)
            bb.instructions = out


def build_nc(fix_waits=True):
    nc = bass.Bass()
    zx = nc.dram_tensor("zx", [128, PW * PW], BF16, kind="ExternalInput")
    wconv = nc.dram_tensor("wconv", [128, KK * 27], BF16, kind="ExternalInput")
    wflat = nc.dram_tensor("wflat", [C, 3 * 3 * O], BF16, kind="ExternalInput")
    bias27 = nc.dram_tensor("bias27", [27, 1], F32, kind="ExternalInput")
    ident = nc.dram_tensor("ident", [32, 32], F32, kind="ExternalInput")
    ident128 = nc.dram_tensor("ident128", [128, 128], BF16, kind="ExternalInput")
    consts = nc.dram_tensor("consts", [128, 6], F32, kind="ExternalInput")
    # shift staging: 134 rows x (576 cols x SLOTS); rows 0-2 / 131-133 stay
    # host-zeroed forever (projections of zero-pad x columns)
    stg = nc.dram_tensor("stg", [PW, 576 * SLOTS], BF16, kind="ExternalInput")
    outT = nc.dram_tensor("outT", [H, W, O], F32, kind="ExternalOutput")

    zx3 = zx.rearrange("p (a b) -> p a b", b=PW)

    with tile.TileContext(nc) as tc:
        with tc.tile_pool(name="persist", bufs=1) as pp:
            WC = pp.tile([128, KK, 27], BF16, tag="wc")
            WF = pp.tile([128, 3 * 3 * O], BF16, tag="wf")  # [c,(kx,ky,o)] p64-127
            BIA = pp.tile([27, 1], F32, tag="bia")
            IDT = pp.tile([32, 32], F32, tag="idt")
            I128 = pp.tile([128, 128], BF16, tag="i128")
            CST = pp.tile([128, 6], F32, tag="cst")  # cols: [-2,-1,0,1,2, 1.0]
            OMT = pp.tile([128, 27, H], BF16, tag="omt")  # conv outs T: [j, plane, i]
            nc.sync.dma_start(WC[:], wconv.rearrange("p (t q) -> p t q", q=27))
            nc.sync.dma_start(WF[64:128, :], wflat[:])
            nc.sync.dma_start(BIA[:], bias27[:])
            nc.sync.dma_start(IDT[:], ident[:])
            nc.sync.dma_start(I128[:], ident128[:])
            nc.sync.dma_start(CST[:], consts[:])

            # convs (phase 1) + transposes (phase 2) are emitted per-band,
            # one band ahead, inside the phase-3 loop: rolling 4-row conv
            # buffer instead of a full-image OM tile, so band 0's MAC starts
            # ~20us in rather than after the whole conv pass.
            with (
                tc.tile_pool(name="zrows", bufs=2) as pzr,
                tc.tile_pool(name="omb", bufs=1) as pomb,
                tc.tile_pool(name="psc", bufs=1, space="PSUM") as pconv,
                tc.tile_pool(name="pst", bufs=1, space="PSUM") as ptr,
                tc.tile_pool(name="vtp", bufs=2) as pvtile,
                tc.tile_pool(name="xrows", bufs=1) as pxr,
                tc.tile_pool(name="band", bufs=1) as pb,
                tc.tile_pool(name="accs", bufs=2) as pacc,
                tc.tile_pool(name="tmps", bufs=2) as ptmp,
                tc.tile_pool(name="tmpd3", bufs=3) as ptmpd,
                tc.tile_pool(name="psv", bufs=2, space="PSUM") as pvt,
                tc.tile_pool(name="psa", bufs=2, space="PSUM") as pac2,
            ):

                def conv_transpose_rows(ib2):
                    # conv + transpose for band ib2's 8 output rows
                    for nt in (2 * ib2, 2 * ib2 + 1):
                        r0 = nt * 4
                        ZB = pzr.tile([128, 7, PW], BF16, tag="zb")
                        nc.sync.dma_start(ZB[:], zx3[:, r0 + 2 : r0 + 9, :])
                        ps = pconv.tile([27, 512], F32, tag="convps")
                        for t in range(KK):
                            ty, tx = t // 3, t % 3
                            rhs = ZB[:, ty : ty + 4, 2 + tx : 2 + tx + W]
                            nc.tensor.matmul(
                                ps[:], WC[:, t, :], rhs,
                                start=(t == 0), stop=(t == KK - 1),
                            )
                        ps3 = ps[:].rearrange("p (a b) -> p a b", b=W)
                        OMB = pomb.tile([27, 4, W], BF16, tag="omb")
                        nc.scalar.activation(
                            OMB[:], ps3, ACTF.Identity, bias=BIA[:, 0:1]
                        )
                        for ri in range(4):
                            pt = ptr.tile([128, 27], BF16, tag="trps")
                            nc.tensor.transpose(
                                pt[:], OMB[:, ri, :], I128[0:27, 0:27]
                            )
                            nc.scalar.copy(OMT[:, :, r0 + ri], pt[:])

                G00 = GOFF[(0, 0)]  # base col of the unshifted kx-triplet
                conv_transpose_rows(0)
                for ib in range(NBANDS):
                    i0 = ib * R
                    # x rows for this band's projections (partitions 64-127)
                    XB = pxr.tile([128, SLOTS, PW], BF16, tag="xb")
                    nc.scalar.dma_start(
                        XB[64:128, :, :], zx3[64:128, i0 : i0 + SLOTS, :]
                    )
                    # unshifted projection (sx=0): VT2[j; col, slot],
                    # col = GOFF(sx,kx)+ky*64+o
                    VT2 = pvtile.tile([128, GW, SLOTS], BF16, tag="vt2")
                    for s in range(SLOTS):
                        ps = pvt.tile([128, 576], F32, tag="vtps")
                        lhsT = XB[64:128, s, 3 : 3 + W]
                        nc.tensor.matmul(
                            ps[:, 0:512], lhsT, WF[64:128, 0:512],
                            start=True, stop=True,
                        )
                        nc.tensor.matmul(
                            ps[:, 512:576], lhsT, WF[64:128, 512:576],
                            start=True, stop=True,
                        )
                        nc.scalar.copy(VT2[:, G00 : G00 + 576, s], ps[:])
                    # shifted variants: stage base group to DRAM, read back
                    # with row offsets (all SBUF APs full-partition; edge rows
                    # of stg are host-zeroed)
                    nc.sync.dma_start(
                        stg[3 : 3 + W, :],
                        VT2[:, G00 : G00 + 576, :].rearrange("p a b -> p (a b)"),
                    )
                    for sx in range(-3, 4):
                        if sx == 0:
                            continue
                        kxlo, kxhi = SXRANGE[sx]
                        ncols = (kxhi - kxlo + 1) * 3 * O
                        g0 = GOFF[(sx, kxlo)]
                        c0 = kxlo * 3 * O * SLOTS
                        dq = nc.sync if sx in (-2, 1, 3) else nc.scalar
                        dq.dma_start(
                            VT2[:, g0 : g0 + ncols, :].rearrange("p a b -> p (a b)"),
                            stg[3 + sx : 3 + sx + W, c0 : c0 + ncols * SLOTS],
                        )

                    # next band's conv/transpose rows (PE slack before psum-adds)
                    if ib + 1 < NBANDS:
                        conv_transpose_rows(ib + 1)

                    # tents for this band (bf16 outputs)
                    dyS = OMT[:, 0:9, i0 : i0 + R]
                    dxS = OMT[:, 9:18, i0 : i0 + R]
                    mskS = pb.tile([128, KK, R], BF16, tag="msk")
                    nc.scalar.activation(
                        mskS[:], OMT[:, 18:27, i0 : i0 + R], ACTF.Sigmoid
                    )
                    TYA = pb.tile([128, 5, KK, R], BF16, tag="tya")
                    TY = pb.tile([128, 5, KK, R], BF16, tag="ty")
                    TYM = pb.tile([128, 5, KK, R], BF16, tag="tym")
                    TXA = pb.tile([128, KK, 5, R], BF16, tag="txa")
                    TX = pb.tile([128, KK, 5, R], BF16, tag="tx")
                    CT = pb.tile([128, KK, 5, 5, R], BF16, tag="ct")
                    one = CST[:, 5:6]
                    for e in range(5):
                        nege = CST[:, 4 - e : 5 - e]  # == -(e-2)
                        nc.scalar.activation(TYA[:, e], dyS, ACTF.Abs, bias=nege)
                        nc.scalar.activation(
                            TY[:, e], TYA[:, e], ACTF.Relu, bias=one, scale=-1.0
                        )
                        nc.scalar.activation(TXA[:, :, e, :], dxS, ACTF.Abs, bias=nege)
                        nc.scalar.activation(
                            TX[:, :, e, :], TXA[:, :, e, :], ACTF.Relu,
                            bias=one, scale=-1.0,
                        )
                    nc.vector.scalar_tensor_tensor(
                        out=TYM[:], in0=TY[:], scalar=2.0,
                        in1=mskS[:, None, :, :].broadcast_to([128, 5, KK, R]),
                        op0=ALU.mult, op1=ALU.mult,
                    )
                    for e in range(5):
                        nc.vector.tensor_tensor(
                            out=CT[:, :, e, :, :],
                            in0=TYM[:, e, :, None, :].broadcast_to([128, KK, 5, R]),
                            in1=TX[:], op=ALU.mult,
                        )

                    # accumulators: psum fp32 (PE terms) + bf16 (DVE terms)
                    ACC2 = pac2.tile([128, O, R], F32, tag="acc2")
                    ACCD = pacc.tile([128, 3, O, R], BF16, tag="accd")
                    ACCF = pacc.tile([128, O, R], F32, tag="accf")
                    state = dict(first_dve_add=True, first_pe=True)

                    def emit_term(kx, ey, ex, stop_pe=False):
                        sx = kx - 1 + ex
                        on_gps = (kx, ey, ex) in GPS_MULT
                        pool_t = ptmp if on_gps else ptmpd
                        tmp = pool_t.tile(
                            [128, 3, O, R], BF16, tag="tmpg" if on_gps else "tmpd"
                        )
                        meng = nc.gpsimd if on_gps else nc.vector
                        # merged 3-ky op via raw AP (ky advances col by O and
                        # slot by 1 jointly); safe: every VT2 write is an
                        # engine copy or a full-partition staging DMA
                        vap = VT2[:]
                        vsl = bass.AP(
                            tensor=vap.tensor,
                            offset=vap.offset + GOFF[(sx, kx)] * SLOTS + (ey + 2),
                            ap=[[GW * SLOTS, 128], [O * SLOTS + 1, 3],
                                [SLOTS, O], [1, R]],
                        )
                        cap_ = CT[:]
                        csl = bass.AP(
                            tensor=cap_.tensor,
                            offset=cap_.offset + kx * 25 * R + (ey + 2) * 5 * R
                            + (ex + 2) * R,
                            ap=[[KK * 25 * R, 128], [75 * R, 3], [0, O], [1, R]],
                        )
                        meng.tensor_tensor(out=tmp[:], in0=vsl, in1=csl, op=ALU.mult)
                        if (kx, ey, ex) in DVE_ADD:
                            if state["first_dve_add"]:
                                nc.vector.tensor_copy(out=ACCD[:], in_=tmp[:])
                                state["first_dve_add"] = False
                            else:
                                nc.vector.tensor_tensor(
                                    out=ACCD[:], in0=ACCD[:], in1=tmp[:], op=ALU.add
                                )
                        else:
                            for kyp in range(3):
                                nc.tensor.matmul(
                                    ACC2[:].rearrange("p a b -> p (a b)"),
                                    I128[:],
                                    tmp[:, kyp].rearrange("p a b -> p (a b)"),
                                    start=(state["first_pe"] and kyp == 0),
                                    stop=(stop_pe and kyp == 2),
                                )
                            state["first_pe"] = False

                    pe_terms = [t for t in TERMS2 if t not in DVE_ADD]
                    for t in TERMS2:
                        emit_term(*t, stop_pe=(t == pe_terms[-1]))

                    # collapse: ACCD ky-planes (bf16), then fp32 merge with psum
                    nc.vector.tensor_tensor(
                        out=ACCD[:, 0], in0=ACCD[:, 0], in1=ACCD[:, 1], op=ALU.add
                    )
                    nc.vector.tensor_tensor(
                        out=ACCD[:, 0], in0=ACCD[:, 0], in1=ACCD[:, 2], op=ALU.add
                    )
                    nc.vector.tensor_tensor(
                        out=ACCF[:], in0=ACCD[:, 0], in1=ACC2[:], op=ALU.add
                    )
                    nc.sync.dma_start(
                        outT[i0 : i0 + R].rearrange("i j o -> j o i"), ACCF[:]
                    )

    if fix_waits:
        _fix_multiwait(nc)
    return nc


def make_consts(w_off, b_off, w_mod, b_mod, w_reg):
    wconv = np.zeros((128, KK, 27), np.float32)
    for t in range(KK):
        ty, tx = t // 3, t % 3
        wconv[0:64, t, 0:18] = w_off[:, :, ty, tx].T     # z half -> offsets
        wconv[64:128, t, 18:27] = w_mod[:, :, ty, tx].T  # x half -> mask
    # reorder offset channels so planes are [dy*9, dx*9, mask*9]
    perm = list(range(0, 18, 2)) + list(range(1, 18, 2)) + list(range(18, 27))
    wconv = wconv[:, :, perm].reshape(128, KK * 27)
    # wflat[c, kx*192 + ky*64 + o] = w_reg[o, c, ky*3+kx]
    w3 = w_reg.reshape(O, C, 3, 3)  # [o, c, ky, kx]
    wflat = np.ascontiguousarray(
        w3.transpose(1, 3, 2, 0).reshape(C, 3 * 3 * O)
    )
    bias27 = np.concatenate([b_off[perm[:18]], b_mod]).reshape(27, 1).astype(
        np.float32
    )
    ident = np.eye(32, dtype=np.float32)
    ident128 = np.eye(128, dtype=np.float32)
    consts = np.tile(
        np.array([-2.0, -1.0, 0.0, 1.0, 2.0, 1.0], np.float32), (128, 1)
    )
    return (
        wconv.astype(np.float32),
        wflat.astype(np.float32),
        bias27,
        ident,
        ident128,
        consts,
    )


def make_zx(z_img, x_img):
    zp = np.zeros((64, PW, PW), np.float32)
    zp[:, 3 : 3 + H, 3 : 3 + W] = z_img
    xp = np.zeros((64, PW, PW), np.float32)
    xp[:, 3 : 3 + H, 3 : 3 + W] = x_img
    return np.concatenate([zp, xp], axis=0).reshape(128, PW * PW)


_NC_CACHE = None


def _get_nc():
    global _NC_CACHE
    if _NC_CACHE is None:
        _NC_CACHE = build_nc()
    return _NC_CACHE


def _make_in_maps(inp):
    import ml_dtypes

    x = np.asarray(inp["x"], np.float32)
    z = np.asarray(inp["z"], np.float32)
    wconv, wflat, bias27, ident, ident128, consts = make_consts(
        np.asarray(inp["w_off"], np.float32), np.asarray(inp["b_off"], np.float32),
        np.asarray(inp["w_mod"], np.float32), np.asarray(inp["b_mod"], np.float32),
        np.asarray(inp["w_reg"], np.float32),
    )
    bf = ml_dtypes.bfloat16
    stgz = np.zeros((PW, 576 * SLOTS), np.float32)
    in_maps = []
    for b in range(x.shape[0]):
        in_maps.append(
            dict(
                zx=make_zx(z[b], x[b]).astype(bf),
                wconv=wconv.astype(bf),
                wflat=wflat.astype(bf),
                bias27=bias27,
                ident=ident,
                ident128=ident128.astype(bf),
                consts=consts,
                stg=stgz.astype(bf),
            )
        )
    return in_maps


def _postprocess_one(outT):
    return np.transpose(outT, (2, 0, 1)).astype(np.float32)


def kernel(x, z, w_off, b_off, w_mod, b_mod, w_reg):
    in_maps = _make_in_maps(
        dict(x=x, z=z, w_off=w_off, b_off=b_off, w_mod=w_mod, b_mod=b_mod,
             w_reg=w_reg)
    )
    nc = _get_nc()
    res = run_bass_kernel_spmd(nc, in_maps, list(range(NCORES)))
    out = np.stack(
        [_postprocess_one(res.results[b]["outT"]) for b in range(len(in_maps))]
    ).astype(np.float32)
    return out
